# revision 10
# baseline (speedup 1.0000x reference)
"""Trainium2 Bass kernel for nn_DiffMPC2 (100-step diagonal-QP SGD recursion).

The reference iterates  u <- u - LR*(2*q*u + p)  100 times, i.e. the affine
per-element map  u <- a*u + b  with  a = 1 - 0.02*q,  b = -0.01*p.  Closed
form:  u_100 = P*u0 + T*p  with  P = a^100,  T = (P - 1)/(2q).

Key algebraic identity:  P = 1 + 2q*T  exactly, so with E = -T >= 0:

    u = u0 - E * (2q*u0 + p),      E = (1 - P)/(2q),

which is smooth on [0,1] (E(0)=1: the reciprocal and its small-q
cancellation disappear from the dataflow entirely -- q=0 is exact).

2*E(q) is approximated by a single LUT evaluation (max rel err 5.4e-3,
measured end-to-end norm rel err 4.1e-3 vs the f64 reference, gate 2e-2):

    2*E(q) ~= -K * ln(S*q + B)      K=0.93394, S=0.28088, B=0.11614

The -K post-scale folds into host-side preprocessing (ship qp = K*q and
pp = (K/2)*p; the Ln input scale becomes S/K), and the sign flip turns the
final subtract into an add, so the whole kernel is:

    Ep = Ln((S/K)*qp + B)                       [ACT, 1 op/elem]
    v1 = qp*u0; v2 = v1+pp; m = Ep*v2; u = u0+m [DVE, 4 x tensor_tensor]

Everything -- I/O and intermediates -- is fp16: halves HBM traffic to
4 MB/core (3 MB in + 1 MB out ~= 11.2 us at the 358 GB/s per-core HBM
limit) and unlocks DVE 2x_1p mode for all four tensor_tensor ops.
Per-core engine busy: ACT ~5.8 us, DVE ~10 us; the kernel is bound by the
HBM stream plus fixed walrus prologue/epilogue (~9 us of the measured
window is framework sem-sweep/barrier overhead we cannot remove).

DMA layout: inputs host-packed PER CHUNK -- [qp_c | pp_c | u0_c] contiguous
per partition -- so each chunk's input DMA is a single 6*w-byte run per
partition (near line-rate).  All DMAs (inputs up front, stores as chunks
complete) issue from the sync HWDGE queue; per-engine FIFO drains inputs
in chunk order, then stores.  Only the LAST store carries the completion
semaphore (ring FIFO makes it imply the others); non-final stores inc a
dump sem nobody waits on (walrus requires a sem per dynamic DMA).

Raw bass (explicit per-engine programs + semaphores).  Sharding: pure data
parallel, batch split across 8 cores; 131072 rows x 4 ctrl cols per core
laid out [128, 4096] fp16.  x_init and the first 12 columns of Q/p are
dead.
"""

import sys

for _p in (
    "/root/.axon_site",
    "/root/.axon_site/_ro/trn_rl_repo",
    "/root/.axon_site/_ro/pypackages",
):
    if _p not in sys.path:
        sys.path.append(_p)

import numpy as np

from concourse import bass, mybir
from concourse.bass_utils import run_bass_kernel_spmd

N_CORES = 8
B = 1048576
S_DIM = 12
C_DIM = 4
PARTS = 128
F_TOTAL = (B // N_CORES) * C_DIM // PARTS  # 4096
# Small first chunk shrinks pipeline fill; big middle chunks amortize
# per-instruction overhead; smaller tail chunk shrinks the drain (last
# DVE + last store after the input stream ends).
CHUNKS = [256, 768, 1024, 1024, 896, 128]
assert sum(CHUNKS) == F_TOTAL
N_CHUNKS = len(CHUNKS)
OFFS = [sum(CHUNKS[:i]) for i in range(N_CHUNKS)]

# Minimax fit  2*E(q) ~= -K*ln(S*q + B)  on [0,1], max rel err 5.35e-3.
K_FIT = 0.9339420518
LN_SCALE = 0.3007474171  # S / K
LN_BIAS = 0.1161437173  # B

_nc_cache = None


def _build_bass():
    f16 = mybir.dt.float16
    f32 = mybir.dt.float32
    Alu = mybir.AluOpType
    Act = mybir.ActivationFunctionType

    nc = bass.Bass()

    # Register the activation-bias constant (Bass only pre-registers 0/1).
    const_memsets = []
    for val in (LN_BIAS,):
        t = nc.alloc_sbuf_tensor(f"const-f32-{val}", [128, 1], f32)
        const_memsets.append(nc.gpsimd.memset(t.ap(), val))
        nc.const_aps.aps[(f32, val)] = t.ap()

    # Packed input, per-chunk contiguous: [qp_c | pp_c | u0_c] per partition.
    xin = nc.declare_dram_parameter("xin", [PARTS, 3 * F_TOTAL], f16, isOutput=False)
    uo = nc.declare_dram_parameter("uo", [PARTS, F_TOTAL], f16, isOutput=True)

    def sb(name, cols):
        return nc.alloc_sbuf_tensor(name, [PARTS, cols], f16).ap()

    tin = sb("tin", 3 * F_TOTAL)

    def in_slices(c):
        b0 = 3 * OFFS[c]
        w = CHUNKS[c]
        tq = tin[:, b0 : b0 + w]
        tp = tin[:, b0 + w : b0 + 2 * w]
        tu = tin[:, b0 + 2 * w : b0 + 3 * w]
        return tq, tp, tu

    # Full-width intermediates, chunk-sliced: disjoint columns, so no
    # cross-chunk hazards and no slot-reuse gating anywhere.
    tE = sb("tE", F_TOTAL)
    tv1 = sb("tv1", F_TOTAL)
    tv2 = sb("tv2", F_TOTAL)
    tm = sb("tm", F_TOTAL)
    tout = sb("tout", F_TOTAL)

    # Per-DMA input semaphores, each waited at its final value (16): a
    # single cumulative sem is racy with several DMAs in flight.
    s_in = [nc.alloc_semaphore(f"s_in{c}") for c in range(N_CHUNKS)]
    # Dump sem for store DMAs whose completion nobody waits on (walrus
    # requires every dynamic DMA to carry a sem update).
    s_junk = nc.alloc_semaphore("s_junk")

    with (
        nc.Block() as block,
        nc.semaphore("s_const") as s_const,
        nc.semaphore("s_act") as s_act,
        nc.semaphore("s_dve") as s_dve,
        nc.semaphore("s_out") as s_out,
    ):
        for ms in const_memsets:
            ms.then_inc(s_const, 1)

        @block.sync
        def _(sp):
            # All input DMAs up front on the qSP HWDGE queue; the per-engine
            # rings drain them in chunk order, then the stores.
            for c in range(N_CHUNKS):
                b0 = 3 * OFFS[c]
                sp.dma_start(
                    out=tin[:, b0 : b0 + 3 * CHUNKS[c]],
                    in_=xin.ap()[:, b0 : b0 + 3 * CHUNKS[c]],
                ).then_inc(s_in[c], 16)
            for c in range(N_CHUNKS):
                sl = slice(OFFS[c], OFFS[c] + CHUNKS[c])
                sp.wait_ge(s_dve, c + 1)
                sp.dma_start(out=uo.ap()[:, sl], in_=tout[:, sl]).then_inc(
                    s_out if c == N_CHUNKS - 1 else s_junk, 16
                )
            sp.wait_ge(s_out, 16)

        @block.scalar
        def _(act):
            # Warm the Ln activation-table set (~1.3us load) while the first
            # input DMA is in flight; scale=0 makes the dummy op
            # input-independent.
            act.wait_ge(s_const, len(const_memsets))
            act.activation(tE[:, :1], tv1[:, :1], Act.Ln, bias=LN_BIAS, scale=0.0)
            for c in range(N_CHUNKS):
                tq, _, _ = in_slices(c)
                sl = slice(OFFS[c], OFFS[c] + CHUNKS[c])
                act.wait_ge(s_in[c], 16)
                act.activation(
                    tE[:, sl], tq, Act.Ln, bias=LN_BIAS, scale=LN_SCALE
                ).then_inc(s_act, 1)

        @block.vector
        def _(v):
            for c in range(N_CHUNKS):
                tq, tp, tu = in_slices(c)
                sl = slice(OFFS[c], OFFS[c] + CHUNKS[c])
                # s_act implies s_in[c] (ACT waited it before the Ln), so one
                # wait per chunk gates all four reads.
                v.wait_ge(s_act, c + 1)
                v.tensor_mul(tv1[:, sl], tq, tu)
                v.tensor_add(tv2[:, sl], tv1[:, sl], tp)
                v.tensor_mul(tm[:, sl], tE[:, sl], tv2[:, sl])
                v.tensor_add(tout[:, sl], tu, tm[:, sl]).then_inc(s_dve, 1)

    return nc


def _get_nc():
    global _nc_cache
    if _nc_cache is None:
        _nc_cache = _build_bass()
    return _nc_cache


def _prep_in_maps(Q, p, u_init):
    q_u = (Q[:, S_DIM:] * np.float32(K_FIT)).astype(np.float16).reshape(
        N_CORES, PARTS, F_TOTAL
    )
    p_u = (p[:, S_DIM:] * np.float32(0.5 * K_FIT)).astype(np.float16).reshape(
        N_CORES, PARTS, F_TOTAL
    )
    u0 = u_init.astype(np.float16).reshape(N_CORES, PARTS, F_TOTAL)
    xin = np.empty((N_CORES, PARTS, 3 * F_TOTAL), dtype=np.float16)
    for c in range(N_CHUNKS):
        b0, w = 3 * OFFS[c], CHUNKS[c]
        sl = slice(OFFS[c], OFFS[c] + w)
        xin[:, :, b0 : b0 + w] = q_u[:, :, sl]
        xin[:, :, b0 + w : b0 + 2 * w] = p_u[:, :, sl]
        xin[:, :, b0 + 2 * w : b0 + 3 * w] = u0[:, :, sl]
    return [{"xin": xin[c]} for c in range(N_CORES)]


def kernel(x_init, Q, p, u_init):
    assert Q.shape == (B, S_DIM + C_DIM) and u_init.shape == (B, C_DIM)
    nc = _get_nc()
    in_maps = _prep_in_maps(Q, p, u_init)
    res = run_bass_kernel_spmd(nc, in_maps, list(range(N_CORES)))
    out = np.stack([res.results[c]["uo"] for c in range(N_CORES)])
    return out.reshape(B, C_DIM).astype(np.float32)


# revision 11
# speedup vs baseline: 1.2319x; 1.2319x over previous
"""Trainium2 Bass kernel for nn_DiffMPC2 (100-step diagonal-QP SGD recursion).

The reference iterates  u <- u - LR*(2*q*u + p)  100 times, i.e. the affine
per-element map  u <- a*u + b  with  a = 1 - 0.02*q,  b = -0.01*p.  Closed
form:  u_100 = P*u0 + T*p  with  P = a^100,  T = (P - 1)/(2q).

Key algebraic identity:  P = 1 + 2q*T  exactly, so with E = -T >= 0:

    u = u0 - E * (2q*u0 + p),      E = (1 - P)/(2q),

which is smooth on [0,1] (E(0)=1: the reciprocal and its small-q
cancellation disappear from the dataflow entirely -- q=0 is exact).

2*E(q) is approximated by a single LUT evaluation (max rel err 5.4e-3,
measured end-to-end norm rel err 4.1e-3 vs the f64 reference, gate 2e-2):

    2*E(q) ~= -K * ln(S*q + B)      K=0.93394, S=0.28088, B=0.11614

The -K post-scale folds into host-side preprocessing (ship qp = K*q and
pp = (K/2)*p; the Ln input scale becomes S/K), and the sign flip turns the
final subtract into an add, so the whole kernel is:

    Ep = Ln((S/K)*qp + B)                       [ACT, 1 op/elem]
    v1 = qp*u0; v2 = v1+pp; m = Ep*v2; u = u0+m [DVE, 4 x tensor_tensor]

Everything -- I/O and intermediates -- is fp16: halves HBM traffic to
4 MB/core (3 MB in + 1 MB out ~= 11.2 us at the 358 GB/s per-core HBM
limit) and unlocks DVE 2x_1p mode for all four tensor_tensor ops.
Per-core engine busy: ACT ~5.8 us, DVE ~10 us; the kernel is bound by the
HBM stream plus fixed walrus prologue/epilogue (~9 us of the measured
window is framework sem-sweep/barrier overhead we cannot remove).

DMA layout: inputs host-packed PER CHUNK -- [qp_c | pp_c | u0_c] contiguous
per partition -- so each chunk's input DMA is a single 6*w-byte run per
partition (near line-rate).  All DMAs (inputs up front, stores as chunks
complete) issue from the sync HWDGE queue; per-engine FIFO drains inputs
in chunk order, then stores.  Only the LAST store carries the completion
semaphore (ring FIFO makes it imply the others); non-final stores inc a
dump sem nobody waits on (walrus requires a sem per dynamic DMA).

Raw bass (explicit per-engine programs + semaphores).  Sharding: pure data
parallel, batch split across 8 cores; 131072 rows x 4 ctrl cols per core
laid out [128, 4096] fp16.  x_init and the first 12 columns of Q/p are
dead.
"""

import sys

for _p in (
    "/root/.axon_site",
    "/root/.axon_site/_ro/trn_rl_repo",
    "/root/.axon_site/_ro/pypackages",
):
    if _p not in sys.path:
        sys.path.append(_p)

import numpy as np

from concourse import bass, mybir
from concourse.bass_utils import run_bass_kernel_spmd

N_CORES = 8
B = 1048576
S_DIM = 12
C_DIM = 4
PARTS = 128
F_TOTAL = (B // N_CORES) * C_DIM // PARTS  # 4096
# Small first chunk shrinks pipeline fill; big middle chunks amortize
# per-instruction overhead; smaller tail chunk shrinks the drain (last
# DVE + last store after the input stream ends).
CHUNKS = [256, 512, 1024, 1024, 896, 384]
assert sum(CHUNKS) == F_TOTAL
N_CHUNKS = len(CHUNKS)
OFFS = [sum(CHUNKS[:i]) for i in range(N_CHUNKS)]

# Minimax fit  2*E(q) ~= -K*ln(S*q + B)  on [0,1], max rel err 5.35e-3.
K_FIT = 0.9339420518
LN_SCALE = 0.3007474171  # S / K
LN_BIAS = 0.1161437173  # B

_nc_cache = None


def _build_bass():
    f16 = mybir.dt.float16
    f32 = mybir.dt.float32
    Alu = mybir.AluOpType
    Act = mybir.ActivationFunctionType

    nc = bass.Bass()

    # Register the activation-bias constant (Bass only pre-registers 0/1).
    const_memsets = []
    for val in (LN_BIAS,):
        t = nc.alloc_sbuf_tensor(f"const-f32-{val}", [128, 1], f32)
        const_memsets.append(nc.gpsimd.memset(t.ap(), val))
        nc.const_aps.aps[(f32, val)] = t.ap()

    # Packed input, per-chunk contiguous: [qp_c | pp_c | u0_c] per partition.
    xin = nc.declare_dram_parameter("xin", [PARTS, 3 * F_TOTAL], f16, isOutput=False)
    uo = nc.declare_dram_parameter("uo", [PARTS, F_TOTAL], f16, isOutput=True)

    def sb(name, cols):
        return nc.alloc_sbuf_tensor(name, [PARTS, cols], f16).ap()

    tin = sb("tin", 3 * F_TOTAL)

    def in_slices(c):
        b0 = 3 * OFFS[c]
        w = CHUNKS[c]
        tq = tin[:, b0 : b0 + w]
        tp = tin[:, b0 + w : b0 + 2 * w]
        tu = tin[:, b0 + 2 * w : b0 + 3 * w]
        return tq, tp, tu

    # Full-width intermediates, chunk-sliced: disjoint columns, so no
    # cross-chunk hazards and no slot-reuse gating anywhere.
    tE = sb("tE", F_TOTAL)
    tv1 = sb("tv1", F_TOTAL)
    tv2 = sb("tv2", F_TOTAL)
    tm = sb("tm", F_TOTAL)
    tout = sb("tout", F_TOTAL)

    # Per-DMA input semaphores, each waited at its final value (16): a
    # single cumulative sem is racy with several DMAs in flight.
    s_in = [nc.alloc_semaphore(f"s_in{c}") for c in range(N_CHUNKS)]
    # Dump sem for store DMAs whose completion nobody waits on (walrus
    # requires every dynamic DMA to carry a sem update).
    s_junk = nc.alloc_semaphore("s_junk")

    with (
        nc.Block() as block,
        nc.semaphore("s_const") as s_const,
        nc.semaphore("s_act") as s_act,
        nc.semaphore("s_dve") as s_dve,
        nc.semaphore("s_out") as s_out,
    ):
        for ms in const_memsets:
            ms.then_inc(s_const, 1)

        @block.sync
        def _(sp):
            # All input DMAs up front on the qSP HWDGE queue; the per-engine
            # rings drain them in chunk order, then the stores.
            for c in range(N_CHUNKS):
                b0 = 3 * OFFS[c]
                sp.dma_start(
                    out=tin[:, b0 : b0 + 3 * CHUNKS[c]],
                    in_=xin.ap()[:, b0 : b0 + 3 * CHUNKS[c]],
                ).then_inc(s_in[c], 16)
            for c in range(N_CHUNKS):
                sl = slice(OFFS[c], OFFS[c] + CHUNKS[c])
                sp.wait_ge(s_dve, c + 1)
                sp.dma_start(out=uo.ap()[:, sl], in_=tout[:, sl]).then_inc(
                    s_out if c == N_CHUNKS - 1 else s_junk, 16
                )
            sp.wait_ge(s_out, 16)

        @block.scalar
        def _(act):
            # Warm the Ln activation-table set (~1.3us load) while the first
            # input DMA is in flight; scale=0 makes the dummy op
            # input-independent.
            act.wait_ge(s_const, len(const_memsets))
            act.activation(tE[:, :1], tv1[:, :1], Act.Ln, bias=LN_BIAS, scale=0.0)
            for c in range(N_CHUNKS):
                tq, _, _ = in_slices(c)
                sl = slice(OFFS[c], OFFS[c] + CHUNKS[c])
                act.wait_ge(s_in[c], 16)
                act.activation(
                    tE[:, sl], tq, Act.Ln, bias=LN_BIAS, scale=LN_SCALE
                ).then_inc(s_act, 1)

        @block.vector
        def _(v):
            for c in range(N_CHUNKS):
                tq, tp, tu = in_slices(c)
                sl = slice(OFFS[c], OFFS[c] + CHUNKS[c])
                v.wait_ge(s_in[c], 16)
                v.tensor_mul(tv1[:, sl], tq, tu)
                v.tensor_add(tv2[:, sl], tv1[:, sl], tp)
                v.wait_ge(s_act, c + 1)
                v.tensor_mul(tm[:, sl], tE[:, sl], tv2[:, sl])
                v.tensor_add(tout[:, sl], tu, tm[:, sl]).then_inc(s_dve, 1)

    return nc


def _get_nc():
    global _nc_cache
    if _nc_cache is None:
        _nc_cache = _build_bass()
    return _nc_cache


def _prep_in_maps(Q, p, u_init):
    q_u = (Q[:, S_DIM:] * np.float32(K_FIT)).astype(np.float16).reshape(
        N_CORES, PARTS, F_TOTAL
    )
    p_u = (p[:, S_DIM:] * np.float32(0.5 * K_FIT)).astype(np.float16).reshape(
        N_CORES, PARTS, F_TOTAL
    )
    u0 = u_init.astype(np.float16).reshape(N_CORES, PARTS, F_TOTAL)
    xin = np.empty((N_CORES, PARTS, 3 * F_TOTAL), dtype=np.float16)
    for c in range(N_CHUNKS):
        b0, w = 3 * OFFS[c], CHUNKS[c]
        sl = slice(OFFS[c], OFFS[c] + w)
        xin[:, :, b0 : b0 + w] = q_u[:, :, sl]
        xin[:, :, b0 + w : b0 + 2 * w] = p_u[:, :, sl]
        xin[:, :, b0 + 2 * w : b0 + 3 * w] = u0[:, :, sl]
    return [{"xin": xin[c]} for c in range(N_CORES)]


def kernel(x_init, Q, p, u_init):
    assert Q.shape == (B, S_DIM + C_DIM) and u_init.shape == (B, C_DIM)
    nc = _get_nc()
    in_maps = _prep_in_maps(Q, p, u_init)
    res = run_bass_kernel_spmd(nc, in_maps, list(range(N_CORES)))
    out = np.stack([res.results[c]["uo"] for c in range(N_CORES)])
    return out.reshape(B, C_DIM).astype(np.float32)
